# revision 4
# baseline (speedup 1.0000x reference)
"""Gaussian-biased attention TRN2 kernel.

B=8 batches sharded across 8 NeuronCores (one batch per core, all 8 heads
on-core).  Per core, for each head:
  scores^ = (Q/8) @ K^T            (fp32r matmuls, PSUM fp32 accumulate)
  g       = (j + 4096*(1-m) - Pi)^2    (ACT Square, mask folded into iota)
  x       = scores - g/9               (DVE scalar_tensor_tensor)
  e       = exp(x), S = rowsum(e)      (ACT Exp with accum_out)
  p       = e / S  -> p_attn           (GpSimd tensor_scalar)
  meT     = e^T (PE transposes, fp32r) ; out^T = V^T @ meT (fp32r)
  out     = (out^T)^T / S
The Pi-MLP (tanh(QW1+b1)W2+b2 -> sigmoid*L) runs in exact fp32 on PE.
"""
import numpy as np

B, H, L, D = 8, 8, 1024, 64
N_CORES = 8
NQT = L // 128          # 8 query tiles of 128
MASK_OFF = 4096.0

_cache = {}


def _build():
    import concourse.bass as bass
    import concourse.tile as tile
    from concourse import bacc, mybir

    F32 = mybir.dt.float32
    F32R = mybir.dt.float32r
    I32 = mybir.dt.int32
    AO = mybir.AluOpType
    AF = mybir.ActivationFunctionType
    PSUM = bass.MemorySpace.PSUM

    nc = bacc.Bacc("TRN2", target_bir_lowering=False, debug=False,
                   num_devices=N_CORES)

    q_d = nc.dram_tensor("query", [H, L, D], F32, kind="ExternalInput").ap()
    k_d = nc.dram_tensor("key", [H, L, D], F32, kind="ExternalInput").ap()
    v_d = nc.dram_tensor("value", [H, L, D], F32, kind="ExternalInput").ap()
    mask_d = nc.dram_tensor("mask", [L, L], I32, kind="ExternalInput").ap()
    adj_d = nc.dram_tensor("adj", [L - 1, L - 1], I32, kind="ExternalInput").ap()
    w1_d = nc.dram_tensor("w1aug", [D + 1, D], F32, kind="ExternalInput").ap()
    w2_d = nc.dram_tensor("w2aug", [D + 1, 1], F32, kind="ExternalInput").ap()
    id_d = nc.dram_tensor("ident", [128, 128], F32, kind="ExternalInput").ap()
    out_d = nc.dram_tensor("out", [H, L, D], F32, kind="ExternalOutput").ap()
    p_d = nc.dram_tensor("p_attn", [H, L, L], F32, kind="ExternalOutput").ap()

    with tile.TileContext(nc) as tc:
        with tc.tile_pool(name="cst", bufs=1) as cst, \
             tc.tile_pool(name="persist", bufs=1) as pers:
            # constants
            idf = cst.tile([128, 128], F32, tag="idf")
            nc.sync.dma_start(idf[:], id_d[:])
            idr = cst.tile([128, 128], F32R, tag="idr")
            nc.vector.tensor_copy(idr[:], idf[:])
            w1s = cst.tile([D + 1, D], F32, tag="w1")
            nc.sync.dma_start(w1s[:], w1_d[:])
            w2s = cst.tile([D + 1, 1], F32, tag="w2")
            nc.sync.dma_start(w2s[:], w2_d[:])
            iota_i = cst.tile([128, L], I32, tag="ioi")
            nc.gpsimd.iota(iota_i[:], pattern=[[1, L]], base=int(MASK_OFF),
                           channel_multiplier=0)
            iota4 = cst.tile([128, L], F32, tag="iof")
            nc.gpsimd.tensor_copy(iota4[:], iota_i[:])

            # per-core mask prep: iotam[qt] = j + 4096 - 4096*mask*adjp
            iotam = [pers.tile([128, L], F32, tag=f"iotam{t}", name=f"iotam{t}")
                     for t in range(NQT)]
            with tc.tile_pool(name="mprep", bufs=2) as mp:
                for t in range(NQT):
                    mrow = mp.tile([128, L], I32, tag="mrow")
                    nc.sync.dma_start(mrow[:], mask_d[t * 128:(t + 1) * 128, :])
                    madj = mp.tile([128, L], I32, tag="madj")
                    nc.gpsimd.memset(madj[:], 1)
                    rows = min(128, (L - 1) - t * 128)
                    if rows > 0:
                        nc.sync.dma_start(
                            madj[0:rows, 0:L - 1],
                            adj_d[t * 128:t * 128 + rows, :])
                    mrf = mp.tile([128, L], F32, tag="mrf")
                    nc.gpsimd.tensor_copy(mrf[:], mrow[:])
                    maf = mp.tile([128, L], F32, tag="maf")
                    nc.gpsimd.tensor_copy(maf[:], madj[:])
                    tmp = mp.tile([128, L], F32, tag="mtmp")
                    nc.vector.scalar_tensor_tensor(
                        tmp[:], mrf[:], -MASK_OFF, maf[:], AO.mult, AO.mult)
                    nc.vector.tensor_tensor(
                        out=iotam[t][:], in0=tmp[:], in1=iota4[:], op=AO.add)

            with tc.tile_pool(name="ld", bufs=2) as ld, \
                 tc.tile_pool(name="tr", bufs=2) as tr, \
                 tc.tile_pool(name="mlp", bufs=2) as mlp, \
                 tc.tile_pool(name="big", bufs=2) as big, \
                 tc.tile_pool(name="ebig", bufs=2) as ebig, \
                 tc.tile_pool(name="met", bufs=1) as metp, \
                 tc.tile_pool(name="sml", bufs=3) as sml, \
                 tc.tile_pool(name="ps_s", bufs=4, space=PSUM) as ps_s, \
                 tc.tile_pool(name="ps_pv", bufs=2, space=PSUM) as ps_pv, \
                 tc.tile_pool(name="ps_q", bufs=2, space=PSUM) as ps_q:
                for h in range(H):
                    # ---- loads ----
                    qn = ld.tile([128, NQT, D], F32, tag="qn")
                    kn = ld.tile([128, NQT, D], F32, tag="kn")
                    vn = ld.tile([128, NQT, D], F32, tag="vn")
                    for t in range(NQT):
                        nc.sync.dma_start(qn[:, t, :], q_d[h, t * 128:(t + 1) * 128, :])
                        nc.sync.dma_start(kn[:, t, :], k_d[h, t * 128:(t + 1) * 128, :])
                        nc.sync.dma_start(vn[:, t, :], v_d[h, t * 128:(t + 1) * 128, :])
                    vr = ld.tile([128, NQT, D], F32R, tag="vr")
                    nc.vector.tensor_copy(vr[:], vn[:])

                    # ---- transposes of Q,K -> [64, 1024] ----
                    qtr = tr.tile([D, L], F32R, tag="qtr")     # Q^T / 8
                    qtaug = tr.tile([D + 1, L], F32, tag="qta")  # [Q^T; 1]
                    ktr = tr.tile([D, L], F32R, tag="ktr")
                    nc.gpsimd.memset(qtaug[D:D + 1, :], 1.0)
                    for t in range(NQT):
                        pq = ps_q.tile([D, 128], F32, tag="pq")
                        nc.tensor.transpose(pq[:], qn[:, t, :], idf[:])
                        nc.vector.tensor_scalar_mul(
                            qtr[:, t * 128:(t + 1) * 128], pq[:], 0.125)
                        nc.scalar.copy(qtaug[0:D, t * 128:(t + 1) * 128], pq[:])
                        pk = ps_q.tile([D, 128], F32, tag="pq")
                        nc.tensor.transpose(pk[:], kn[:, t, :], idf[:])
                        nc.vector.tensor_copy(ktr[:, t * 128:(t + 1) * 128], pk[:])

                    # ---- Pi MLP (exact fp32) ----
                    htaug = mlp.tile([D + 1, L], F32, tag="hta")
                    nc.gpsimd.memset(htaug[D:D + 1, :], 1.0)
                    for half in range(2):
                        sl = slice(half * 512, (half + 1) * 512)
                        ph = ps_pv.tile([D, 512], F32, tag="pv")
                        nc.tensor.matmul(ph[:], w1s[:], qtaug[:, sl],
                                         start=True, stop=True)
                        nc.scalar.activation(htaug[0:D, sl], ph[:], AF.Tanh,
                                             bias=0.0, scale=1.0)
                    negpi_row = mlp.tile([1, L], F32, tag="npr")
                    for half in range(2):
                        sl = slice(half * 512, (half + 1) * 512)
                        pz = ps_pv.tile([1, 512], F32, tag="pv")
                        nc.tensor.matmul(pz[:], w2s[:], htaug[:, sl],
                                         start=True, stop=True)
                        sg = mlp.tile([1, 512], F32, tag="sg")
                        nc.scalar.activation(sg[:], pz[:], AF.Sigmoid,
                                             bias=0.0, scale=1.0)
                        nc.vector.tensor_scalar_mul(
                            negpi_row[:, sl], sg[:], -float(L))
                    npc_ps = ps_q.tile([128, NQT], F32, tag="pq")
                    for t in range(NQT):
                        nc.tensor.transpose(
                            npc_ps[:, t:t + 1],
                            negpi_row[0:1, t * 128:(t + 1) * 128],
                            idf[0:1, 0:1])
                    negpi = mlp.tile([128, NQT], F32, tag="npc_sb")
                    nc.vector.tensor_copy(negpi[:], npc_ps[:])

                    S_all = sml.tile([128, NQT], F32, tag="sall")
                    recip = sml.tile([128, NQT], F32, tag="recip")
                    meT = metp.tile([128, NQT, L], F32R, tag="meT")

                    # ---- per q-tile main pipeline ----
                    for t in range(NQT):
                        g = big.tile([128, L], F32, tag="g")
                        nc.scalar.activation(g[:], iotam[t][:], AF.Square,
                                             bias=negpi[:, t:t + 1], scale=1.0)
                        x = big.tile([128, L], F32, tag="x")
                        for half in range(2):
                            sl = slice(half * 512, (half + 1) * 512)
                            sp = ps_s.tile([128, 512], F32, tag="s")
                            nc.tensor.matmul(
                                sp[:], qtr[:, t * 128:(t + 1) * 128],
                                ktr[:, sl], start=True, stop=True)
                            nc.vector.scalar_tensor_tensor(
                                x[:, sl], g[:, sl], -1.0 / 9.0, sp[:],
                                AO.mult, AO.add)
                        e = ebig.tile([128, L], F32, tag="e")
                        nc.scalar.activation(e[:], x[:], AF.Exp,
                                             bias=0.0, scale=1.0,
                                             accum_out=S_all[:, t:t + 1])
                        nc.vector.reciprocal(recip[:, t:t + 1], S_all[:, t:t + 1])
                        p = big.tile([128, L], F32, tag="p")
                        nc.gpsimd.tensor_scalar(
                            p[:], e[:], recip[:, t:t + 1], None, AO.mult)
                        nc.sync.dma_start(p_d[h, t * 128:(t + 1) * 128, :], p[:])
                        # transposes of e -> meT (fp32r), grouped 4 per psum bank
                        for grp in range(2):
                            tp = ps_s.tile([128, 512], F32, tag="s")
                            for c in range(4):
                                jt = grp * 4 + c
                                nc.tensor.transpose(
                                    tp[:, c * 128:(c + 1) * 128],
                                    e[:, jt * 128:(jt + 1) * 128], idf[:])
                            dst = meT[:, grp * 4:(grp + 1) * 4,
                                      t * 128:(t + 1) * 128]
                            src = tp[:].rearrange("p (c f) -> p c f", c=4)
                            if grp == 0:
                                nc.vector.tensor_copy(dst, src)
                            else:
                                nc.scalar.copy(dst, src)

                    # ---- PV: out^T = V^T @ meT ----
                    for qh in range(2):
                        pv = ps_pv.tile([D, 512], F32, tag="pv")
                        for jt in range(NQT):
                            nc.tensor.matmul(
                                pv[:], vr[:, jt, :],
                                meT[:, jt, qh * 512:(qh + 1) * 512],
                                start=(jt == 0), stop=(jt == NQT - 1))
                        pvs = sml.tile([D, 512], F32R, tag="pvs")
                        nc.vector.tensor_copy(pvs[:], pv[:])
                        for c in range(4):
                            t = qh * 4 + c
                            ot = ps_q.tile([128, D], F32R, tag="pq")
                            nc.tensor.transpose(
                                ot[:], pvs[:, c * 128:(c + 1) * 128],
                                idr[0:D, 0:D])
                            of = sml.tile([128, D], F32, tag="of")
                            nc.vector.tensor_scalar_mul(
                                of[:], ot[:], recip[:, t:t + 1])
                            nc.sync.dma_start(
                                out_d[h, t * 128:(t + 1) * 128, :], of[:])

    nc.compile()
    return nc


def _get_nc():
    if "nc" not in _cache:
        _cache["nc"] = _build()
    return _cache["nc"]


def kernel(query, key, value, mask, adj, W1, b1, W2, b2):
    from concourse.bass_utils import run_bass_kernel_spmd

    nc = _get_nc()
    w1aug = np.concatenate([W1, b1[None, :]], axis=0).astype(np.float32)
    w2aug = np.concatenate([W2, b2[None, :]], axis=0).astype(np.float32)
    ident = np.eye(128, dtype=np.float32)
    in_maps = []
    for b in range(B):
        in_maps.append({
            "query": np.ascontiguousarray(query[b]),
            "key": np.ascontiguousarray(key[b]),
            "value": np.ascontiguousarray(value[b]),
            "mask": np.ascontiguousarray(mask[b]),
            "adj": np.ascontiguousarray(adj[b]),
            "w1aug": w1aug, "w2aug": w2aug, "ident": ident,
        })
    res = run_bass_kernel_spmd(nc, in_maps, list(range(N_CORES))).results
    out = np.stack([res[b]["out"] for b in range(B)])
    p = np.stack([res[b]["p_attn"] for b in range(B)])
    return out, p
